# revision 78
# baseline (speedup 1.0000x reference)
"""Dual cross-attention + 1x1 conv kernel for Trainium2 (8 NeuronCores).

Sharding: each core handles one (sample, query-half) pair: B=4 samples x 2
halves = 8 cores.  Attention is per-sample, so cores are fully independent
(pure SPMD, no collectives).  Keys/values span the full 4096 pixels of the
sample; the key order is permuted per core so each core's own query half
always sits at columns [0, 2048) -- softmax sums are order-invariant.

Per-core program (identical on all cores, data differs):
  conv phase:  Q/K/V^T 1x1 convs (bf16 matmuls), biases added via K=1
               ones-matmuls, PSUM->SBUF copies on ACT/DVE.
  attention:   for each 512-query chunk, for each group of 4 key-tiles:
               QK^T matmuls write bf16 logits to PSUM (2 banks/group,
               double buffered), one Exp activation per group
               (scale=C^-0.5 folded in, no max-subtraction -- logits are
               tiny by construction), then AV matmuls accumulate
               V^T.T @ E into fp32 PSUM while ones-matmuls with E as the
               stationary operand accumulate the softmax denominators,
               transposed (queries on partitions) for a cheap reciprocal.
  tail:        reciprocal on DVE, DMA round-trip through DRAM to
               transpose+broadcast 1/D, projection matmuls, then
               out = (P * (1/D) + (wp@bv + bp)) + residual on DVE.
"""

import numpy as np
import ml_dtypes
from contextlib import ExitStack

import concourse.bass as bass
import concourse.tile as tile
from concourse import bacc, mybir
from concourse.bass_utils import run_bass_kernel_spmd

F32 = mybir.dt.float32
BF16 = mybir.dt.bfloat16
EXP = mybir.ActivationFunctionType.Exp
ADD = mybir.AluOpType.add
MULT = mybir.AluOpType.mult

C2D, C3D = 256, 128
IA, IB = 128, 64
B, H, W = 4, 64, 64
N = H * W          # 4096 pixels per sample (keys per core)
NQ = N // 2        # 2048 queries per core
CHW = 512          # queries per chunk
NCH = NQ // CHW    # 4 chunks
MT = 128           # keys per m-tile
NMT = N // MT      # 32 m-tiles
GR = 4             # m-tiles per exp group (log tile = GR banks, single buffer)
NGR = NMT // GR    # 8 groups
NJ = CHW // 128    # 4 query sub-blocks per chunk (for denominators)


def _ap(t):
    return t.ap() if hasattr(t, "ap") else t


def _declare_io(nc):
    io = {}

    def inp(name, shape, dt):
        io[name] = nc.dram_tensor(name, list(shape), dt, kind="ExternalInput").ap()

    def outp(name, shape, dt):
        io[name] = nc.dram_tensor(name, list(shape), dt, kind="ExternalOutput").ap()

    inp("f2d_bf", [C2D, N], BF16)
    inp("f3d_bf", [C3D, N], BF16)
    inp("f2d_res", [C2D, NQ], F32)
    inp("f3d_res", [C3D, NQ], F32)
    inp("wqA_T", [C2D, IA], BF16)
    inp("wkA_T", [C3D, IA], BF16)
    inp("wvA_T", [C3D, IA], BF16)
    inp("wpA_T", [IA, C2D], BF16)
    inp("wqB_T", [C3D, IB], BF16)
    inp("wkB_T", [C2D, IB], BF16)
    inp("wvB_T", [C2D, IB], BF16)
    inp("wpB_T", [IB, C3D], BF16)
    inp("bqA_r", [1, IA], BF16)
    inp("bkA_r", [1, IA], BF16)
    inp("bqB_r", [1, IB], BF16)
    inp("bkB_r", [1, IB], BF16)
    inp("cA", [C2D], F32)
    inp("cB", [C3D], F32)
    outp("x2d", [C2D, NQ], F32)
    outp("x3d", [C3D, NQ], F32)
    return io


def _emit(ctx, tc, io, with_bias):
    nc = tc.nc
    mm = nc.tensor.matmul

    const = ctx.enter_context(tc.tile_pool(name="const", bufs=1))

    # ---- constants into SBUF ------------------------------------------------
    # weights/biases ride the Scalar HWDGE queue so the big feature loads on
    # the Sync queue aren't stuck behind a dozen small transfers
    def cload(name, shape, dt, src, eng=None):
        t = const.tile(list(shape), dt, name=name)
        (eng or nc.scalar).dma_start(out=t[:], in_=src)
        return t

    wkA = cload("wkA", [128, IA], BF16, io["wkA_T"])
    wvA = cload("wvA", [128, IA], BF16, io["wvA_T"])
    wqA = cload("wqA", [128, 2, IA], BF16,
                io["wqA_T"].rearrange("(a p) m -> p a m", a=2))
    wpA = cload("wpA", [128, C2D], BF16, io["wpA_T"])
    wqB = cload("wqB", [128, IB], BF16, io["wqB_T"])
    wkB = cload("wkB", [128, 2, IB], BF16,
                io["wkB_T"].rearrange("(a p) m -> p a m", a=2))
    wvB = cload("wvB", [128, 2, IB], BF16,
                io["wvB_T"].rearrange("(a p) m -> p a m", a=2))
    wpB = cload("wpB", [64, C3D], BF16, io["wpB_T"])

    if with_bias:
        bqA = cload("bqA", [1, IA], BF16, io["bqA_r"])
        bkA = cload("bkA", [1, IA], BF16, io["bkA_r"])
        bqB = cload("bqB", [1, IB], BF16, io["bqB_r"])
        bkB = cload("bkB", [1, IB], BF16, io["bkB_r"])
    else:
        bqA = bkA = bqB = bkB = None
    cA = cload("cA", [128, 2], F32, io["cA"].rearrange("(a p) -> p a", a=2))
    cB = cload("cB", [128, 1], F32, io["cB"].unsqueeze(1))

    # features: f3d first (feeds the first convs), residuals deferred below;
    # loaded in halves so the first conv groups start before the full load
    f3d_b = const.tile([128, N], BF16, name="f3d_b")
    f2d_b = const.tile([128, 2, N], BF16, name="f2d_b")
    f2d_src = io["f2d_bf"].rearrange("(a p) m -> p a m", a=2)
    for h in range(2):
        cs_ = slice(h * 2048, (h + 1) * 2048)
        nc.sync.dma_start(out=f3d_b[:, cs_], in_=io["f3d_bf"][:, cs_])
    for h in range(2):
        cs_ = slice(h * 2048, (h + 1) * 2048)
        nc.sync.dma_start(out=f2d_b[:, :, cs_], in_=f2d_src[:, :, cs_])

    ones_row = const.tile([1, CHW], BF16, name="ones_row")
    nc.vector.memset(ones_row[:], 1.0)
    # all-ones stationary folds the tree root across partitions into a
    # denominator replicated on all 128 partitions (broadcast for free)
    ones_mat = const.tile([128, 128], BF16, name="ones_mat")
    nc.vector.memset(ones_mat[:], 1.0)

    # ---- Q/K/V^T conv phase -------------------------------------------------
    qa = const.tile([128, NQ], BF16, name="qa")
    ka = const.tile([128, N], BF16, name="ka")
    vta = const.tile([128, NMT, 128], BF16, name="vta")
    qb = const.tile([64, NQ], BF16, name="qb")
    kb = const.tile([64, N], BF16, name="kb")
    # 65th column = ones: the AV matmul then emits softmax denominators
    # for stream B as PSUM row 64, for free (cost is N cycles regardless of M)
    vtb = const.tile([128, NMT, 65], BF16, name="vtb")
    nc.vector.memset(vtb[:, :, 64:65], 1.0)

    def conv_group(psum, j, steps, bias):
        steps = list(steps)
        if bias is not None:
            steps.append((bias, ones_row))
        for i, (lh, rh) in enumerate(steps):
            mm(psum[:, j, :], lhsT=lh, rhs=rh,
               start=(i == 0), stop=(i == len(steps) - 1))

    with tc.tile_pool(name="convp", bufs=2, space="PSUM") as convp:
        # f3d-fed convs first (f3d_b DMA lands before f2d_b)
        # K_A = wkA @ f3d + bkA -> [128, 4096]  (2 groups of 4 chunks)
        for g0 in range(2):
            gk = convp.tile([128, 4, CHW], F32, name="cvk", tag="cv")
            for j in range(4):
                s = slice((g0 * 4 + j) * CHW, (g0 * 4 + j + 1) * CHW)
                conv_group(gk, j, [(wkA[:], f3d_b[:, s])], bkA)
            nc.scalar.copy(
                out=ka[:, g0 * 2048:(g0 + 1) * 2048].rearrange("p (a b) -> p a b", a=4),
                in_=gk[:])

        # V_A^T[m, c] = f3d^T @ wvA^T -> [128, 32, 128]
        for g0 in range(2):
            gv = convp.tile([128, 16, 128], F32, name="cvv", tag="cv")
            for t in range(16):
                tm = g0 * 16 + t
                mm(gv[:, t, :], lhsT=f3d_b[:, tm * MT:(tm + 1) * MT], rhs=wvA[:],
                   start=True, stop=True)
            nc.vector.tensor_copy(out=vta[:, g0 * 16:(g0 + 1) * 16, :], in_=gv[:])

        # Q_B = wqB @ f3d[:, :NQ] + bqB -> [64, 2048]
        gq = convp.tile([64, 4, CHW], F32, name="cvqb", tag="cv")
        for j in range(4):
            s = slice(j * CHW, (j + 1) * CHW)
            conv_group(gq, j, [(wqB[:], f3d_b[:, s])], bqB)
        nc.scalar.copy(out=qb.rearrange("p (a b) -> p a b", a=4), in_=gq[:])

        # Q_A = wqA @ f2d[:, :NQ] + bqA  -> [128, 2048]
        g = convp.tile([128, 4, CHW], F32, name="cvq", tag="cv")
        for j in range(4):
            s = slice(j * CHW, (j + 1) * CHW)
            conv_group(g, j, [(wqA[:, 0, :], f2d_b[:, 0, s]),
                              (wqA[:, 1, :], f2d_b[:, 1, s])], bqA)
        nc.scalar.copy(out=qa.rearrange("p (a b) -> p a b", a=4), in_=g[:])

        # K_B = wkB @ f2d + bkB -> [64, 4096]
        for g0 in range(2):
            gk = convp.tile([64, 4, CHW], F32, name="cvkb", tag="cv")
            for j in range(4):
                s = slice((g0 * 4 + j) * CHW, (g0 * 4 + j + 1) * CHW)
                conv_group(gk, j, [(wkB[:, 0, :], f2d_b[:, 0, s]),
                                   (wkB[:, 1, :], f2d_b[:, 1, s])], bkB)
            nc.vector.tensor_copy(
                out=kb[:, g0 * 2048:(g0 + 1) * 2048].rearrange("p (a b) -> p a b", a=4),
                in_=gk[:])

        # V_B^T part is below; residual feature loads are only needed at
        # chunk tails, so queue them behind the conv-critical DMAs
        f2d_r = cload("f2d_r", [128, 2, NQ], F32,
                      io["f2d_res"].rearrange("(a p) m -> p a m", a=2),
                      eng=nc.sync)
        f3d_r = cload("f3d_r", [128, NQ], F32, io["f3d_res"], eng=nc.sync)

        # V_B^T[m, c] = f2d^T @ wvB^T -> [128, 32, 64]
        for g0 in range(2):
            gv = convp.tile([128, 16, 128], F32, name="cvvb", tag="cv")
            for t in range(16):
                tm = g0 * 16 + t
                ms = slice(tm * MT, (tm + 1) * MT)
                mm(gv[:, t, 0:64], lhsT=f2d_b[:, 0, ms], rhs=wvB[:, 0, :], start=True, stop=False)
                mm(gv[:, t, 0:64], lhsT=f2d_b[:, 1, ms], rhs=wvB[:, 1, :], start=False, stop=True)
            nc.vector.tensor_copy(out=vtb[:, g0 * 16:(g0 + 1) * 16, 0:64],
                                  in_=gv[:, :, 0:64])

    # ---- attention main loop ------------------------------------------------
    log_pool = ctx.enter_context(tc.tile_pool(name="plog", bufs=1, space="PSUM"))
    o_pool = ctx.enter_context(tc.tile_pool(name="pacc", bufs=2, space="PSUM"))
    pj_pool = ctx.enter_context(tc.tile_pool(name="ppj", bufs=2, space="PSUM"))
    e_pool = ctx.enter_context(tc.tile_pool(name="epool", bufs=4))
    wk_pool = ctx.enter_context(tc.tile_pool(name="wk", bufs=3))
    dr_pool = ctx.enter_context(tc.tile_pool(name="drs", bufs=4, space="DRAM"))

    streams = [
        dict(cs=IA, aug=False, scale=float(IA) ** -0.5, K=ka, Q=qa, VT=vta,
             WP=wpA, cpart=cA, res=f2d_r, out=io["x2d"], nproj=2),
        dict(cs=IB, aug=True, scale=float(IB) ** -0.5, K=kb, Q=qb, VT=vtb,
             WP=wpB, cpart=cB, res=f3d_r, out=io["x3d"], nproj=1),
    ]

    # Global software pipeline over (chunk, group): the AV lag carries across
    # chunk boundaries, so chunk c's last AV groups interleave with chunk
    # c+1's first QK/exp groups instead of draining + refilling the pipe.
    state = [dict(st=st, pend=[], o_ci=None) for st in streams]

    def emit_qk(s, ci, g):
        st = s["st"]
        n0 = ci * CHW
        lg = log_pool.tile([128, GR, CHW], F32, name="lg", tag="lg")
        for t in range(GR):
            tm = g * GR + t
            mm(lg[:, t, :], lhsT=st["K"][:, tm * MT:(tm + 1) * MT],
               rhs=st["Q"][:, n0:n0 + CHW], start=True, stop=True)
        E = e_pool.tile([128, GR, CHW], BF16, name="E", tag="E")
        # two half-tile activations so bank-level deps let the next QK group
        # start overwriting banks as soon as its half is consumed
        nc.scalar.activation(E[:, 0:2, :], lg[:, 0:2, :], EXP, scale=st["scale"])
        nc.scalar.activation(E[:, 2:4, :], lg[:, 2:4, :], EXP, scale=st["scale"])
        s["pend"].append((ci, g, E))

    def emit_av(s):
        st = s["st"]
        mw = st["cs"] + 1 if st["aug"] else st["cs"]
        ci, g, E = s["pend"].pop(0)
        if s["o_ci"] != ci:
            s["o_ci"] = ci
            s["o_ps"] = o_pool.tile([128, CHW], F32, name="o_ps", tag="o")
            s["tree"] = {}
        for t in range(GR):
            tm = g * GR + t
            mm(s["o_ps"][0:mw, :], lhsT=st["VT"][:, tm, 0:mw], rhs=E[:, t, :],
               start=(tm == 0), stop=(tm == NMT - 1))
        if not st["aug"]:
            # stream-A denominators on the otherwise-idle DVE: pairwise bf16
            # adds (2x packed mode) cascading a binary tree over the E tiles
            tree = s["tree"]
            for h in range(GR // 2):
                l1 = wk_pool.tile([128, CHW], BF16, name="l1", tag="tl1")
                nc.vector.tensor_tensor(out=l1[:], in0=E[:, 2 * h, :],
                                        in1=E[:, 2 * h + 1, :], op=ADD)
                tree.setdefault(1, []).append(l1)
                lev = 1
                while len(tree.get(lev, [])) == 2:
                    a, b = tree[lev]
                    tree[lev] = []
                    nxt = wk_pool.tile([128, CHW], BF16, name="lv",
                                       tag=f"tl{lev + 1}")
                    nc.vector.tensor_tensor(out=nxt[:], in0=a[:], in1=b[:],
                                            op=ADD)
                    tree.setdefault(lev + 1, []).append(nxt)
                    lev += 1
        if g == NGR - 1:
            emit_tail(s, ci)

    def emit_tail(s, ci):
        n0 = ci * CHW
        st, o_ps = s["st"], s["o_ps"]
        cs = st["cs"]
        if True:
            # O to SBUF first: frees the PSUM slot for the next chunk before
            # the reciprocal/broadcast chain occupies the DVE
            o_sb = wk_pool.tile([128, CHW], BF16, name="o_sb", tag="osb")
            nc.vector.tensor_copy(out=o_sb[0:cs, :], in_=o_ps[0:cs, :])

            # ---- denominators: reciprocal (+ broadcast via DRAM for B) ----
            bca = wk_pool.tile([128, CHW], F32, name="bca", tag="bca")
            if st["aug"]:
                rec = wk_pool.tile([1, CHW], F32, name="rec", tag="rec")
                nc.vector.reciprocal(rec[:], o_ps[cs:cs + 1, :])
                dsc = dr_pool.tile([1, CHW], F32, name="dsc", tag="dsc")
                nc.sync.dma_start(out=dsc[:], in_=rec[:])
                src = bass.AP(tensor=dsc.tensor, offset=dsc.offset,
                              ap=[[0, 128], [1, CHW]])
                nc.sync.dma_start(out=bca[:], in_=src)
            else:
                # fold the bf16 tree root across all 128 partitions with one
                # all-ones matmul; output rows are all D, so the reciprocal
                # of the tile IS the broadcast 1/D
                d5 = s["tree"][5].pop()
                d_ps = pj_pool.tile([128, CHW], F32, name="d_ps", tag="pj")
                mm(d_ps[:], lhsT=ones_mat[:], rhs=d5[:], start=True, stop=True)
                nc.vector.reciprocal(bca[:], d_ps[:])

            # ---- projection + normalize + residual ----
            for k in range(st["nproj"]):
                p_ps = pj_pool.tile([128, CHW], F32, name="p_ps", tag="pj")
                mm(p_ps[:], lhsT=st["WP"][0:cs, k * 128:(k + 1) * 128],
                   rhs=o_sb[0:cs, :], start=True, stop=True)
                tmp = wk_pool.tile([128, CHW], F32, name="tmp", tag="tmp")
                nc.vector.tensor_tensor(out=tmp[:], in0=p_ps[:], in1=bca[:], op=MULT)
                outt = wk_pool.tile([128, CHW], F32, name="outt", tag="outt")
                if st["nproj"] == 2:
                    resid = st["res"][:, k, n0:n0 + CHW]
                else:
                    resid = st["res"][:, n0:n0 + CHW]
                nc.vector.scalar_tensor_tensor(
                    out=outt[:], in0=tmp[:], scalar=st["cpart"][:, k:k + 1],
                    in1=resid, op0=ADD, op1=ADD)
                nc.sync.dma_start(
                    out=st["out"][k * 128:(k + 1) * 128, n0:n0 + CHW], in_=outt[:])

    for ci in range(NCH):
        for gg in range(0, NGR, 2):
            for s in state:
                emit_qk(s, ci, gg)
                emit_qk(s, ci, gg + 1)
                while len(s["pend"]) > 2:
                    emit_av(s)
    for s in state:
        while s["pend"]:
            emit_av(s)


_BUILT = {}


def _get_built(with_bias=False):
    if with_bias not in _BUILT:
        nc = bacc.Bacc("TRN2", target_bir_lowering=False, debug=False)
        io = _declare_io(nc)
        with tile.TileContext(nc) as tc:
            with ExitStack() as ctx:
                _emit(ctx, tc, io, with_bias)
        nc.compile()
        _BUILT[with_bias] = nc
    return _BUILT[with_bias]


def make_in_maps(features_2d, features_3d,
                 wqA, bqA, wkA, bkA, wvA, bvA, wpA, bpA,
                 wqB, bqB, wkB, bkB, wvB, bvB, wpB, bpB):
    f32 = np.float32
    bf = ml_dtypes.bfloat16
    f2d = np.asarray(features_2d, f32).reshape(B, C2D, N)
    f3d = np.asarray(features_3d, f32).reshape(B, C3D, N)
    wqA, bqA = np.asarray(wqA, f32), np.asarray(bqA, f32)
    wkA, bkA = np.asarray(wkA, f32), np.asarray(bkA, f32)
    wvA, bvA = np.asarray(wvA, f32), np.asarray(bvA, f32)
    wpA, bpA = np.asarray(wpA, f32), np.asarray(bpA, f32)
    wqB, bqB = np.asarray(wqB, f32), np.asarray(bqB, f32)
    wkB, bkB = np.asarray(wkB, f32), np.asarray(bkB, f32)
    wvB, bvB = np.asarray(wvB, f32), np.asarray(bvB, f32)
    wpB, bpB = np.asarray(wpB, f32), np.asarray(bpB, f32)

    shared = {
        "wqA_T": np.ascontiguousarray(wqA.T).astype(bf),
        "wkA_T": np.ascontiguousarray(wkA.T).astype(bf),
        "wvA_T": np.ascontiguousarray(wvA.T).astype(bf),
        "wpA_T": np.ascontiguousarray(wpA.T).astype(bf),
        "wqB_T": np.ascontiguousarray(wqB.T).astype(bf),
        "wkB_T": np.ascontiguousarray(wkB.T).astype(bf),
        "wvB_T": np.ascontiguousarray(wvB.T).astype(bf),
        "wpB_T": np.ascontiguousarray(wpB.T).astype(bf),
        "bqA_r": bqA[None, :].astype(bf),
        "bkA_r": bkA[None, :].astype(bf),
        "bqB_r": bqB[None, :].astype(bf),
        "bkB_r": bkB[None, :].astype(bf),
        "cA": (wpA @ bvA + bpA).astype(f32),
        "cB": (wpB @ bvB + bpB).astype(f32),
    }
    in_maps = []
    for core in range(8):
        b, half = divmod(core, 2)
        if half == 0:
            p2, p3 = f2d[b], f3d[b]
        else:
            p2 = np.concatenate([f2d[b][:, NQ:], f2d[b][:, :NQ]], axis=1)
            p3 = np.concatenate([f3d[b][:, NQ:], f3d[b][:, :NQ]], axis=1)
        in_maps.append({
            "f2d_bf": np.ascontiguousarray(p2).astype(bf),
            "f3d_bf": np.ascontiguousarray(p3).astype(bf),
            "f2d_res": np.ascontiguousarray(p2[:, :NQ]),
            "f3d_res": np.ascontiguousarray(p3[:, :NQ]),
            **shared,
        })
    return in_maps


def assemble(results):
    x2d = np.empty((B, C2D, N), np.float32)
    x3d = np.empty((B, C3D, N), np.float32)
    for core in range(8):
        b, half = divmod(core, 2)
        x2d[b][:, half * NQ:(half + 1) * NQ] = results[core]["x2d"]
        x3d[b][:, half * NQ:(half + 1) * NQ] = results[core]["x3d"]
    return x2d.reshape(B, C2D, H, W), x3d.reshape(B, C3D, H, W)


def kernel(**inputs):
    with_bias = any(
        np.any(np.asarray(inputs[k], np.float32))
        for k in ("bqA", "bkA", "bqB", "bkB"))
    nc = _get_built(with_bias)
    in_maps = make_in_maps(**inputs)
    res = run_bass_kernel_spmd(nc, in_maps, list(range(8))).results
    return assemble(res)


# revision 79
# speedup vs baseline: 1.3692x; 1.3692x over previous
"""Dual cross-attention + 1x1 conv kernel for Trainium2 (8 NeuronCores).

Sharding: each core handles one (sample, query-half) pair: B=4 samples x 2
halves = 8 cores.  Attention is per-sample, so cores are fully independent
(pure SPMD, no collectives).  Keys/values span the full 4096 pixels of the
sample; the key order is permuted per core so each core's own query half
always sits at columns [0, 2048) -- softmax sums are order-invariant.

Per-core program (identical on all cores, data differs):
  conv phase:  Q/K/V^T 1x1 convs (bf16 matmuls), biases added via K=1
               ones-matmuls, PSUM->SBUF copies on ACT/DVE.
  attention:   for each 512-query chunk, for each group of 4 key-tiles:
               QK^T matmuls write bf16 logits to PSUM (2 banks/group,
               double buffered), one Exp activation per group
               (scale=C^-0.5 folded in, no max-subtraction -- logits are
               tiny by construction), then AV matmuls accumulate
               V^T.T @ E into fp32 PSUM while ones-matmuls with E as the
               stationary operand accumulate the softmax denominators,
               transposed (queries on partitions) for a cheap reciprocal.
  tail:        reciprocal on DVE, DMA round-trip through DRAM to
               transpose+broadcast 1/D, projection matmuls, then
               out = (P * (1/D) + (wp@bv + bp)) + residual on DVE.
"""

import numpy as np
import ml_dtypes
from contextlib import ExitStack

import concourse.bass as bass
import concourse.tile as tile
from concourse import bacc, mybir
from concourse.bass_utils import run_bass_kernel_spmd

F32 = mybir.dt.float32
BF16 = mybir.dt.bfloat16
EXP = mybir.ActivationFunctionType.Exp
ADD = mybir.AluOpType.add
MULT = mybir.AluOpType.mult

C2D, C3D = 256, 128
IA, IB = 128, 64
B, H, W = 4, 64, 64
N = H * W          # 4096 pixels per sample (keys per core)
NQ = N // 2        # 2048 queries per core
CHW = 512          # queries per chunk
NCH = NQ // CHW    # 4 chunks
MT = 128           # keys per m-tile
NMT = N // MT      # 32 m-tiles
GR = 2             # m-tiles per exp group (log tile = GR banks, double buffered)
NGR = NMT // GR    # 16 groups
NJ = CHW // 128    # 4 query sub-blocks per chunk (for denominators)


def _ap(t):
    return t.ap() if hasattr(t, "ap") else t


def _declare_io(nc):
    io = {}

    def inp(name, shape, dt):
        io[name] = nc.dram_tensor(name, list(shape), dt, kind="ExternalInput").ap()

    def outp(name, shape, dt):
        io[name] = nc.dram_tensor(name, list(shape), dt, kind="ExternalOutput").ap()

    inp("f2d_bf", [C2D, N], BF16)
    inp("f3d_bf", [C3D, N], BF16)
    inp("f2d_res", [C2D, NQ], F32)
    inp("f3d_res", [C3D, NQ], F32)
    inp("wqA_T", [C2D, IA], BF16)
    inp("wkA_T", [C3D, IA], BF16)
    inp("wvA_T", [C3D, IA], BF16)
    inp("wpA_T", [IA, C2D], BF16)
    inp("wqB_T", [C3D, IB], BF16)
    inp("wkB_T", [C2D, IB], BF16)
    inp("wvB_T", [C2D, IB], BF16)
    inp("wpB_T", [IB, C3D], BF16)
    inp("bqA_r", [1, IA], BF16)
    inp("bkA_r", [1, IA], BF16)
    inp("bqB_r", [1, IB], BF16)
    inp("bkB_r", [1, IB], BF16)
    inp("cA", [C2D], F32)
    inp("cB", [C3D], F32)
    outp("x2d", [C2D, NQ], F32)
    outp("x3d", [C3D, NQ], F32)
    return io


def _emit(ctx, tc, io, with_bias):
    nc = tc.nc
    mm = nc.tensor.matmul

    const = ctx.enter_context(tc.tile_pool(name="const", bufs=1))

    # ---- constants into SBUF ------------------------------------------------
    # weights/biases ride the Scalar HWDGE queue so the big feature loads on
    # the Sync queue aren't stuck behind a dozen small transfers
    def cload(name, shape, dt, src, eng=None):
        t = const.tile(list(shape), dt, name=name)
        (eng or nc.scalar).dma_start(out=t[:], in_=src)
        return t

    wkA = cload("wkA", [128, IA], BF16, io["wkA_T"])
    wvA = cload("wvA", [128, IA], BF16, io["wvA_T"])
    wqA = cload("wqA", [128, 2, IA], BF16,
                io["wqA_T"].rearrange("(a p) m -> p a m", a=2))
    wpA = cload("wpA", [128, C2D], BF16, io["wpA_T"])
    wqB = cload("wqB", [128, IB], BF16, io["wqB_T"])
    wkB = cload("wkB", [128, 2, IB], BF16,
                io["wkB_T"].rearrange("(a p) m -> p a m", a=2))
    wvB = cload("wvB", [128, 2, IB], BF16,
                io["wvB_T"].rearrange("(a p) m -> p a m", a=2))
    wpB = cload("wpB", [64, C3D], BF16, io["wpB_T"])

    if with_bias:
        bqA = cload("bqA", [1, IA], BF16, io["bqA_r"])
        bkA = cload("bkA", [1, IA], BF16, io["bkA_r"])
        bqB = cload("bqB", [1, IB], BF16, io["bqB_r"])
        bkB = cload("bkB", [1, IB], BF16, io["bkB_r"])
    else:
        bqA = bkA = bqB = bkB = None
    cA = cload("cA", [128, 2], F32, io["cA"].rearrange("(a p) -> p a", a=2))
    cB = cload("cB", [128, 1], F32, io["cB"].unsqueeze(1))

    # features: f3d first (feeds the first convs), residuals deferred below;
    # loaded in halves so the first conv groups start before the full load
    f3d_b = const.tile([128, N], BF16, name="f3d_b")
    f2d_b = const.tile([128, 2, N], BF16, name="f2d_b")
    f2d_src = io["f2d_bf"].rearrange("(a p) m -> p a m", a=2)
    for h in range(2):
        cs_ = slice(h * 2048, (h + 1) * 2048)
        nc.sync.dma_start(out=f3d_b[:, cs_], in_=io["f3d_bf"][:, cs_])
    for h in range(2):
        cs_ = slice(h * 2048, (h + 1) * 2048)
        nc.sync.dma_start(out=f2d_b[:, :, cs_], in_=f2d_src[:, :, cs_])

    ones_row = const.tile([1, CHW], BF16, name="ones_row")
    nc.vector.memset(ones_row[:], 1.0)
    # all-ones stationary folds the tree root across partitions into a
    # denominator replicated on all 128 partitions (broadcast for free)
    ones_mat = const.tile([128, 128], BF16, name="ones_mat")
    nc.vector.memset(ones_mat[:], 1.0)

    # ---- Q/K/V^T conv phase -------------------------------------------------
    qa = const.tile([128, NQ], BF16, name="qa")
    ka = const.tile([128, N], BF16, name="ka")
    vta = const.tile([128, NMT, 128], BF16, name="vta")
    qb = const.tile([64, NQ], BF16, name="qb")
    kb = const.tile([64, N], BF16, name="kb")
    # 65th column = ones: the AV matmul then emits softmax denominators
    # for stream B as PSUM row 64, for free (cost is N cycles regardless of M)
    vtb = const.tile([128, NMT, 65], BF16, name="vtb")
    nc.vector.memset(vtb[:, :, 64:65], 1.0)

    def conv_group(psum, j, steps, bias):
        steps = list(steps)
        if bias is not None:
            steps.append((bias, ones_row))
        for i, (lh, rh) in enumerate(steps):
            mm(psum[:, j, :], lhsT=lh, rhs=rh,
               start=(i == 0), stop=(i == len(steps) - 1))

    with tc.tile_pool(name="convp", bufs=2, space="PSUM") as convp:
        # f3d-fed convs first (f3d_b DMA lands before f2d_b)
        # K_A = wkA @ f3d + bkA -> [128, 4096]  (2 groups of 4 chunks)
        for g0 in range(2):
            gk = convp.tile([128, 4, CHW], F32, name="cvk", tag="cv")
            for j in range(4):
                s = slice((g0 * 4 + j) * CHW, (g0 * 4 + j + 1) * CHW)
                conv_group(gk, j, [(wkA[:], f3d_b[:, s])], bkA)
            nc.scalar.copy(
                out=ka[:, g0 * 2048:(g0 + 1) * 2048].rearrange("p (a b) -> p a b", a=4),
                in_=gk[:])

        # V_A^T[m, c] = f3d^T @ wvA^T -> [128, 32, 128]
        for g0 in range(2):
            gv = convp.tile([128, 16, 128], F32, name="cvv", tag="cv")
            for t in range(16):
                tm = g0 * 16 + t
                mm(gv[:, t, :], lhsT=f3d_b[:, tm * MT:(tm + 1) * MT], rhs=wvA[:],
                   start=True, stop=True)
            nc.vector.tensor_copy(out=vta[:, g0 * 16:(g0 + 1) * 16, :], in_=gv[:])

        # Q_B = wqB @ f3d[:, :NQ] + bqB -> [64, 2048]
        gq = convp.tile([64, 4, CHW], F32, name="cvqb", tag="cv")
        for j in range(4):
            s = slice(j * CHW, (j + 1) * CHW)
            conv_group(gq, j, [(wqB[:], f3d_b[:, s])], bqB)
        nc.scalar.copy(out=qb.rearrange("p (a b) -> p a b", a=4), in_=gq[:])

        # Q_A = wqA @ f2d[:, :NQ] + bqA  -> [128, 2048]
        g = convp.tile([128, 4, CHW], F32, name="cvq", tag="cv")
        for j in range(4):
            s = slice(j * CHW, (j + 1) * CHW)
            conv_group(g, j, [(wqA[:, 0, :], f2d_b[:, 0, s]),
                              (wqA[:, 1, :], f2d_b[:, 1, s])], bqA)
        nc.scalar.copy(out=qa.rearrange("p (a b) -> p a b", a=4), in_=g[:])

        # K_B = wkB @ f2d + bkB -> [64, 4096]
        for g0 in range(2):
            gk = convp.tile([64, 4, CHW], F32, name="cvkb", tag="cv")
            for j in range(4):
                s = slice((g0 * 4 + j) * CHW, (g0 * 4 + j + 1) * CHW)
                conv_group(gk, j, [(wkB[:, 0, :], f2d_b[:, 0, s]),
                                   (wkB[:, 1, :], f2d_b[:, 1, s])], bkB)
            nc.vector.tensor_copy(
                out=kb[:, g0 * 2048:(g0 + 1) * 2048].rearrange("p (a b) -> p a b", a=4),
                in_=gk[:])

        # V_B^T part is below; residual feature loads are only needed at
        # chunk tails, so queue them behind the conv-critical DMAs
        f2d_r = cload("f2d_r", [128, 2, NQ], F32,
                      io["f2d_res"].rearrange("(a p) m -> p a m", a=2),
                      eng=nc.sync)
        f3d_r = cload("f3d_r", [128, NQ], F32, io["f3d_res"], eng=nc.sync)

        # V_B^T[m, c] = f2d^T @ wvB^T -> [128, 32, 64]
        for g0 in range(2):
            gv = convp.tile([128, 16, 128], F32, name="cvvb", tag="cv")
            for t in range(16):
                tm = g0 * 16 + t
                ms = slice(tm * MT, (tm + 1) * MT)
                mm(gv[:, t, 0:64], lhsT=f2d_b[:, 0, ms], rhs=wvB[:, 0, :], start=True, stop=False)
                mm(gv[:, t, 0:64], lhsT=f2d_b[:, 1, ms], rhs=wvB[:, 1, :], start=False, stop=True)
            nc.vector.tensor_copy(out=vtb[:, g0 * 16:(g0 + 1) * 16, 0:64],
                                  in_=gv[:, :, 0:64])

    # ---- attention main loop ------------------------------------------------
    log_pool = ctx.enter_context(tc.tile_pool(name="plog", bufs=2, space="PSUM"))
    o_pool = ctx.enter_context(tc.tile_pool(name="pacc", bufs=2, space="PSUM"))
    pj_pool = ctx.enter_context(tc.tile_pool(name="ppj", bufs=2, space="PSUM"))
    e_pool = ctx.enter_context(tc.tile_pool(name="epool", bufs=8))
    wk_pool = ctx.enter_context(tc.tile_pool(name="wk", bufs=3))
    dr_pool = ctx.enter_context(tc.tile_pool(name="drs", bufs=4, space="DRAM"))

    streams = [
        dict(cs=IA, aug=False, scale=float(IA) ** -0.5, K=ka, Q=qa, VT=vta,
             WP=wpA, cpart=cA, res=f2d_r, out=io["x2d"], nproj=2),
        dict(cs=IB, aug=True, scale=float(IB) ** -0.5, K=kb, Q=qb, VT=vtb,
             WP=wpB, cpart=cB, res=f3d_r, out=io["x3d"], nproj=1),
    ]

    # Global software pipeline over (chunk, group): the AV lag carries across
    # chunk boundaries, so chunk c's last AV groups interleave with chunk
    # c+1's first QK/exp groups instead of draining + refilling the pipe.
    state = [dict(st=st, pend=[], o_ci=None) for st in streams]

    def emit_qk(s, ci, g):
        st = s["st"]
        n0 = ci * CHW
        lg = log_pool.tile([128, GR, CHW], F32, name="lg", tag="lg")
        for t in range(GR):
            tm = g * GR + t
            mm(lg[:, t, :], lhsT=st["K"][:, tm * MT:(tm + 1) * MT],
               rhs=st["Q"][:, n0:n0 + CHW], start=True, stop=True)
        E = e_pool.tile([128, GR, CHW], BF16, name="E", tag="E")
        nc.scalar.activation(E[:], lg[:], EXP, scale=st["scale"])
        s["pend"].append((ci, g, E))

    def emit_av(s):
        st = s["st"]
        mw = st["cs"] + 1 if st["aug"] else st["cs"]
        ci, g, E = s["pend"].pop(0)
        if s["o_ci"] != ci:
            s["o_ci"] = ci
            s["o_ps"] = o_pool.tile([128, CHW], F32, name="o_ps", tag="o")
            s["tree"] = {}
        for t in range(GR):
            tm = g * GR + t
            mm(s["o_ps"][0:mw, :], lhsT=st["VT"][:, tm, 0:mw], rhs=E[:, t, :],
               start=(tm == 0), stop=(tm == NMT - 1))
        if not st["aug"]:
            # stream-A denominators on the otherwise-idle DVE: pairwise bf16
            # adds (2x packed mode) cascading a binary tree over the E tiles
            l1 = wk_pool.tile([128, CHW], BF16, name="l1", tag="tl1")
            nc.vector.tensor_tensor(out=l1[:], in0=E[:, 0, :], in1=E[:, 1, :],
                                    op=ADD)
            tree = s["tree"]
            tree.setdefault(1, []).append(l1)
            lev = 1
            while len(tree.get(lev, [])) == 2:
                a, b = tree[lev]
                tree[lev] = []
                nxt = wk_pool.tile([128, CHW], BF16, name="lv",
                                   tag=f"tl{lev + 1}")
                nc.vector.tensor_tensor(out=nxt[:], in0=a[:], in1=b[:], op=ADD)
                tree.setdefault(lev + 1, []).append(nxt)
                lev += 1
        if g == NGR - 1:
            emit_tail(s, ci)

    def emit_tail(s, ci):
        n0 = ci * CHW
        st, o_ps = s["st"], s["o_ps"]
        cs = st["cs"]
        if True:
            # O to SBUF first: frees the PSUM slot for the next chunk before
            # the reciprocal/broadcast chain occupies the DVE
            o_sb = wk_pool.tile([128, CHW], BF16, name="o_sb", tag="osb")
            nc.vector.tensor_copy(out=o_sb[0:cs, :], in_=o_ps[0:cs, :])

            # ---- denominators: reciprocal (+ broadcast via DRAM for B) ----
            bca = wk_pool.tile([128, CHW], F32, name="bca", tag="bca")
            if st["aug"]:
                rec = wk_pool.tile([1, CHW], F32, name="rec", tag="rec")
                nc.vector.reciprocal(rec[:], o_ps[cs:cs + 1, :])
                dsc = dr_pool.tile([1, CHW], F32, name="dsc", tag="dsc")
                nc.sync.dma_start(out=dsc[:], in_=rec[:])
                src = bass.AP(tensor=dsc.tensor, offset=dsc.offset,
                              ap=[[0, 128], [1, CHW]])
                nc.sync.dma_start(out=bca[:], in_=src)
            else:
                # fold the bf16 tree root across all 128 partitions with one
                # all-ones matmul; output rows are all D, so the reciprocal
                # of the tile IS the broadcast 1/D
                d5 = s["tree"][5].pop()
                d_ps = pj_pool.tile([128, CHW], F32, name="d_ps", tag="pj")
                mm(d_ps[:], lhsT=ones_mat[:], rhs=d5[:], start=True, stop=True)
                nc.vector.reciprocal(bca[:], d_ps[:])

            # ---- projection + normalize + residual ----
            for k in range(st["nproj"]):
                p_ps = pj_pool.tile([128, CHW], F32, name="p_ps", tag="pj")
                mm(p_ps[:], lhsT=st["WP"][0:cs, k * 128:(k + 1) * 128],
                   rhs=o_sb[0:cs, :], start=True, stop=True)
                tmp = wk_pool.tile([128, CHW], F32, name="tmp", tag="tmp")
                nc.vector.tensor_tensor(out=tmp[:], in0=p_ps[:], in1=bca[:], op=MULT)
                outt = wk_pool.tile([128, CHW], F32, name="outt", tag="outt")
                if st["nproj"] == 2:
                    resid = st["res"][:, k, n0:n0 + CHW]
                else:
                    resid = st["res"][:, n0:n0 + CHW]
                nc.vector.scalar_tensor_tensor(
                    out=outt[:], in0=tmp[:], scalar=st["cpart"][:, k:k + 1],
                    in1=resid, op0=ADD, op1=ADD)
                nc.sync.dma_start(
                    out=st["out"][k * 128:(k + 1) * 128, n0:n0 + CHW], in_=outt[:])

    for ci in range(NCH):
        for gg in range(0, NGR, 2):
            for s in state:
                emit_qk(s, ci, gg)
                emit_qk(s, ci, gg + 1)
                while len(s["pend"]) > 2:
                    emit_av(s)
    for s in state:
        while s["pend"]:
            emit_av(s)


_BUILT = {}


def _get_built(with_bias=False):
    if with_bias not in _BUILT:
        nc = bacc.Bacc("TRN2", target_bir_lowering=False, debug=False)
        io = _declare_io(nc)
        with tile.TileContext(nc) as tc:
            with ExitStack() as ctx:
                _emit(ctx, tc, io, with_bias)
        nc.compile()
        _BUILT[with_bias] = nc
    return _BUILT[with_bias]


def make_in_maps(features_2d, features_3d,
                 wqA, bqA, wkA, bkA, wvA, bvA, wpA, bpA,
                 wqB, bqB, wkB, bkB, wvB, bvB, wpB, bpB):
    f32 = np.float32
    bf = ml_dtypes.bfloat16
    f2d = np.asarray(features_2d, f32).reshape(B, C2D, N)
    f3d = np.asarray(features_3d, f32).reshape(B, C3D, N)
    wqA, bqA = np.asarray(wqA, f32), np.asarray(bqA, f32)
    wkA, bkA = np.asarray(wkA, f32), np.asarray(bkA, f32)
    wvA, bvA = np.asarray(wvA, f32), np.asarray(bvA, f32)
    wpA, bpA = np.asarray(wpA, f32), np.asarray(bpA, f32)
    wqB, bqB = np.asarray(wqB, f32), np.asarray(bqB, f32)
    wkB, bkB = np.asarray(wkB, f32), np.asarray(bkB, f32)
    wvB, bvB = np.asarray(wvB, f32), np.asarray(bvB, f32)
    wpB, bpB = np.asarray(wpB, f32), np.asarray(bpB, f32)

    shared = {
        "wqA_T": np.ascontiguousarray(wqA.T).astype(bf),
        "wkA_T": np.ascontiguousarray(wkA.T).astype(bf),
        "wvA_T": np.ascontiguousarray(wvA.T).astype(bf),
        "wpA_T": np.ascontiguousarray(wpA.T).astype(bf),
        "wqB_T": np.ascontiguousarray(wqB.T).astype(bf),
        "wkB_T": np.ascontiguousarray(wkB.T).astype(bf),
        "wvB_T": np.ascontiguousarray(wvB.T).astype(bf),
        "wpB_T": np.ascontiguousarray(wpB.T).astype(bf),
        "bqA_r": bqA[None, :].astype(bf),
        "bkA_r": bkA[None, :].astype(bf),
        "bqB_r": bqB[None, :].astype(bf),
        "bkB_r": bkB[None, :].astype(bf),
        "cA": (wpA @ bvA + bpA).astype(f32),
        "cB": (wpB @ bvB + bpB).astype(f32),
    }
    in_maps = []
    for core in range(8):
        b, half = divmod(core, 2)
        if half == 0:
            p2, p3 = f2d[b], f3d[b]
        else:
            p2 = np.concatenate([f2d[b][:, NQ:], f2d[b][:, :NQ]], axis=1)
            p3 = np.concatenate([f3d[b][:, NQ:], f3d[b][:, :NQ]], axis=1)
        in_maps.append({
            "f2d_bf": np.ascontiguousarray(p2).astype(bf),
            "f3d_bf": np.ascontiguousarray(p3).astype(bf),
            "f2d_res": np.ascontiguousarray(p2[:, :NQ]),
            "f3d_res": np.ascontiguousarray(p3[:, :NQ]),
            **shared,
        })
    return in_maps


def assemble(results):
    x2d = np.empty((B, C2D, N), np.float32)
    x3d = np.empty((B, C3D, N), np.float32)
    for core in range(8):
        b, half = divmod(core, 2)
        x2d[b][:, half * NQ:(half + 1) * NQ] = results[core]["x2d"]
        x3d[b][:, half * NQ:(half + 1) * NQ] = results[core]["x3d"]
    return x2d.reshape(B, C2D, H, W), x3d.reshape(B, C3D, H, W)


def kernel(**inputs):
    with_bias = any(
        np.any(np.asarray(inputs[k], np.float32))
        for k in ("bqA", "bkA", "bqB", "bkB"))
    nc = _get_built(with_bias)
    in_maps = make_in_maps(**inputs)
    res = run_bass_kernel_spmd(nc, in_maps, list(range(8))).results
    return assemble(res)


# revision 81
# speedup vs baseline: 1.4104x; 1.0301x over previous
"""Dual cross-attention + 1x1 conv kernel for Trainium2 (8 NeuronCores).

Sharding: each core handles one (sample, query-half) pair: B=4 samples x 2
halves = 8 cores.  Attention is per-sample, so cores are fully independent
(pure SPMD, no collectives).  Keys/values span the full 4096 pixels of the
sample; the key order is permuted per core so each core's own query half
always sits at columns [0, 2048) -- softmax sums are order-invariant.

Per-core program (identical on all cores, data differs):
  conv phase:  Q/K/V^T 1x1 convs (bf16 matmuls), biases added via K=1
               ones-matmuls, PSUM->SBUF copies on ACT/DVE.
  attention:   for each 512-query chunk, for each group of 4 key-tiles:
               QK^T matmuls write bf16 logits to PSUM (2 banks/group,
               double buffered), one Exp activation per group
               (scale=C^-0.5 folded in, no max-subtraction -- logits are
               tiny by construction), then AV matmuls accumulate
               V^T.T @ E into fp32 PSUM while ones-matmuls with E as the
               stationary operand accumulate the softmax denominators,
               transposed (queries on partitions) for a cheap reciprocal.
  tail:        reciprocal on DVE, DMA round-trip through DRAM to
               transpose+broadcast 1/D, projection matmuls, then
               out = (P * (1/D) + (wp@bv + bp)) + residual on DVE.
"""

import numpy as np
import ml_dtypes
from contextlib import ExitStack

import concourse.bass as bass
import concourse.tile as tile
from concourse import bacc, mybir
from concourse.bass_utils import run_bass_kernel_spmd

F32 = mybir.dt.float32
BF16 = mybir.dt.bfloat16
EXP = mybir.ActivationFunctionType.Exp
ADD = mybir.AluOpType.add
MULT = mybir.AluOpType.mult

C2D, C3D = 256, 128
IA, IB = 128, 64
B, H, W = 4, 64, 64
N = H * W          # 4096 pixels per sample (keys per core)
NQ = N // 2        # 2048 queries per core
CHW = 512          # queries per chunk
NCH = NQ // CHW    # 4 chunks
MT = 128           # keys per m-tile
NMT = N // MT      # 32 m-tiles
GR = 2             # m-tiles per exp group (log tile = GR banks, double buffered)
NGR = NMT // GR    # 16 groups
NJ = CHW // 128    # 4 query sub-blocks per chunk (for denominators)


def _ap(t):
    return t.ap() if hasattr(t, "ap") else t


def _declare_io(nc):
    io = {}

    def inp(name, shape, dt):
        io[name] = nc.dram_tensor(name, list(shape), dt, kind="ExternalInput").ap()

    def outp(name, shape, dt):
        io[name] = nc.dram_tensor(name, list(shape), dt, kind="ExternalOutput").ap()

    inp("f2d_bf", [C2D, N], BF16)
    inp("f3d_bf", [C3D, N], BF16)
    inp("f2d_res", [C2D, NQ], F32)
    inp("f3d_res", [C3D, NQ], F32)
    inp("wqA_T", [C2D, IA], BF16)
    inp("wkA_T", [C3D, IA], BF16)
    inp("wvA_T", [C3D, IA], BF16)
    inp("wpA_T", [IA, C2D], BF16)
    inp("wqB_T", [C3D, IB], BF16)
    inp("wkB_T", [C2D, IB], BF16)
    inp("wvB_T", [C2D, IB], BF16)
    inp("wpB_T", [IB, C3D], BF16)
    inp("bqA_r", [1, IA], BF16)
    inp("bkA_r", [1, IA], BF16)
    inp("bqB_r", [1, IB], BF16)
    inp("bkB_r", [1, IB], BF16)
    inp("cA", [C2D], F32)
    inp("cB", [C3D], F32)
    outp("x2d", [C2D, NQ], F32)
    outp("x3d", [C3D, NQ], F32)
    return io


def _emit(ctx, tc, io, with_bias):
    nc = tc.nc
    mm = nc.tensor.matmul

    const = ctx.enter_context(tc.tile_pool(name="const", bufs=1))

    # ---- constants into SBUF ------------------------------------------------
    # weights/biases ride the Scalar HWDGE queue so the big feature loads on
    # the Sync queue aren't stuck behind a dozen small transfers
    def cload(name, shape, dt, src, eng=None):
        t = const.tile(list(shape), dt, name=name)
        (eng or nc.scalar).dma_start(out=t[:], in_=src)
        return t

    wkA = cload("wkA", [128, IA], BF16, io["wkA_T"])
    wvA = cload("wvA", [128, IA], BF16, io["wvA_T"])
    wqA = cload("wqA", [128, 2, IA], BF16,
                io["wqA_T"].rearrange("(a p) m -> p a m", a=2))
    wpA = cload("wpA", [128, C2D], BF16, io["wpA_T"])
    wqB = cload("wqB", [128, IB], BF16, io["wqB_T"])
    wkB = cload("wkB", [128, 2, IB], BF16,
                io["wkB_T"].rearrange("(a p) m -> p a m", a=2))
    wvB = cload("wvB", [128, 2, IB], BF16,
                io["wvB_T"].rearrange("(a p) m -> p a m", a=2))
    wpB = cload("wpB", [64, C3D], BF16, io["wpB_T"])

    if with_bias:
        bqA = cload("bqA", [1, IA], BF16, io["bqA_r"])
        bkA = cload("bkA", [1, IA], BF16, io["bkA_r"])
        bqB = cload("bqB", [1, IB], BF16, io["bqB_r"])
        bkB = cload("bkB", [1, IB], BF16, io["bkB_r"])
    else:
        bqA = bkA = bqB = bkB = None
    cA = cload("cA", [128, 2], F32, io["cA"].rearrange("(a p) -> p a", a=2))
    cB = cload("cB", [128, 1], F32, io["cB"].unsqueeze(1))

    # features: f3d first (feeds the first convs), residuals deferred below;
    # loaded in halves so the first conv groups start before the full load
    f3d_b = const.tile([128, N], BF16, name="f3d_b")
    f2d_b = const.tile([128, 2, N], BF16, name="f2d_b")
    f2d_src = io["f2d_bf"].rearrange("(a p) m -> p a m", a=2)
    for h in range(2):
        cs_ = slice(h * 2048, (h + 1) * 2048)
        nc.sync.dma_start(out=f3d_b[:, cs_], in_=io["f3d_bf"][:, cs_])
    for h in range(2):
        cs_ = slice(h * 2048, (h + 1) * 2048)
        nc.sync.dma_start(out=f2d_b[:, :, cs_], in_=f2d_src[:, :, cs_])

    ones_row = const.tile([1, CHW], BF16, name="ones_row")
    nc.vector.memset(ones_row[:], 1.0)
    # all-ones stationary folds the tree root across partitions into a
    # denominator replicated on all 128 partitions (broadcast for free)
    ones_mat = const.tile([128, 128], BF16, name="ones_mat")
    nc.vector.memset(ones_mat[:], 1.0)

    # ---- Q/K/V^T conv phase -------------------------------------------------
    qa = const.tile([128, NQ], BF16, name="qa")
    ka = const.tile([128, N], BF16, name="ka")
    vta = const.tile([128, NMT, 128], BF16, name="vta")
    qb = const.tile([64, NQ], BF16, name="qb")
    kb = const.tile([64, N], BF16, name="kb")
    # 65th column = ones: the AV matmul then emits softmax denominators
    # for stream B as PSUM row 64, for free (cost is N cycles regardless of M)
    vtb = const.tile([128, NMT, 65], BF16, name="vtb")
    nc.vector.memset(vtb[:, :, 64:65], 1.0)

    def conv_group(psum, j, steps, bias):
        steps = list(steps)
        if bias is not None:
            steps.append((bias, ones_row))
        for i, (lh, rh) in enumerate(steps):
            mm(psum[:, j, :], lhsT=lh, rhs=rh,
               start=(i == 0), stop=(i == len(steps) - 1))

    with tc.tile_pool(name="convp", bufs=2, space="PSUM") as convp:
        # f3d-fed convs first (f3d_b DMA lands before f2d_b)
        # K_A = wkA @ f3d + bkA -> [128, 4096]  (2 groups of 4 chunks)
        for g0 in range(2):
            gk = convp.tile([128, 4, CHW], F32, name="cvk", tag="cv")
            for j in range(4):
                s = slice((g0 * 4 + j) * CHW, (g0 * 4 + j + 1) * CHW)
                conv_group(gk, j, [(wkA[:], f3d_b[:, s])], bkA)
            nc.scalar.copy(
                out=ka[:, g0 * 2048:(g0 + 1) * 2048].rearrange("p (a b) -> p a b", a=4),
                in_=gk[:])

        # V_A^T[m, c] = f3d^T @ wvA^T -> [128, 32, 128]
        for g0 in range(2):
            gv = convp.tile([128, 16, 128], F32, name="cvv", tag="cv")
            for t in range(16):
                tm = g0 * 16 + t
                mm(gv[:, t, :], lhsT=f3d_b[:, tm * MT:(tm + 1) * MT], rhs=wvA[:],
                   start=True, stop=True)
            nc.vector.tensor_copy(out=vta[:, g0 * 16:(g0 + 1) * 16, :], in_=gv[:])

        # Q_B = wqB @ f3d[:, :NQ] + bqB -> [64, 2048]
        gq = convp.tile([64, 4, CHW], F32, name="cvqb", tag="cv")
        for j in range(4):
            s = slice(j * CHW, (j + 1) * CHW)
            conv_group(gq, j, [(wqB[:], f3d_b[:, s])], bqB)
        nc.scalar.copy(out=qb.rearrange("p (a b) -> p a b", a=4), in_=gq[:])

        # Q_A = wqA @ f2d[:, :NQ] + bqA  -> [128, 2048]
        g = convp.tile([128, 4, CHW], F32, name="cvq", tag="cv")
        for j in range(4):
            s = slice(j * CHW, (j + 1) * CHW)
            conv_group(g, j, [(wqA[:, 0, :], f2d_b[:, 0, s]),
                              (wqA[:, 1, :], f2d_b[:, 1, s])], bqA)
        nc.scalar.copy(out=qa.rearrange("p (a b) -> p a b", a=4), in_=g[:])

        # K_B = wkB @ f2d + bkB -> [64, 4096]
        for g0 in range(2):
            gk = convp.tile([64, 4, CHW], F32, name="cvkb", tag="cv")
            for j in range(4):
                s = slice((g0 * 4 + j) * CHW, (g0 * 4 + j + 1) * CHW)
                conv_group(gk, j, [(wkB[:, 0, :], f2d_b[:, 0, s]),
                                   (wkB[:, 1, :], f2d_b[:, 1, s])], bkB)
            nc.vector.tensor_copy(
                out=kb[:, g0 * 2048:(g0 + 1) * 2048].rearrange("p (a b) -> p a b", a=4),
                in_=gk[:])

        # V_B^T part is below; residual feature loads are only needed at
        # chunk tails, so queue them behind the conv-critical DMAs
        f2d_r = cload("f2d_r", [128, 2, NQ], F32,
                      io["f2d_res"].rearrange("(a p) m -> p a m", a=2),
                      eng=nc.sync)
        f3d_r = cload("f3d_r", [128, NQ], F32, io["f3d_res"], eng=nc.sync)

        # V_B^T[m, c] = f2d^T @ wvB^T -> [128, 32, 64]
        for g0 in range(2):
            gv = convp.tile([128, 16, 128], F32, name="cvvb", tag="cv")
            for t in range(16):
                tm = g0 * 16 + t
                ms = slice(tm * MT, (tm + 1) * MT)
                mm(gv[:, t, 0:64], lhsT=f2d_b[:, 0, ms], rhs=wvB[:, 0, :], start=True, stop=False)
                mm(gv[:, t, 0:64], lhsT=f2d_b[:, 1, ms], rhs=wvB[:, 1, :], start=False, stop=True)
            nc.vector.tensor_copy(out=vtb[:, g0 * 16:(g0 + 1) * 16, 0:64],
                                  in_=gv[:, :, 0:64])

    # ---- attention main loop ------------------------------------------------
    log_pool = ctx.enter_context(tc.tile_pool(name="plog", bufs=2, space="PSUM"))
    o_pool = ctx.enter_context(tc.tile_pool(name="pacc", bufs=2, space="PSUM"))
    pj_pool = ctx.enter_context(tc.tile_pool(name="ppj", bufs=2, space="PSUM"))
    e_pool = ctx.enter_context(tc.tile_pool(name="epool", bufs=8))
    wk_pool = ctx.enter_context(tc.tile_pool(name="wk", bufs=3))
    dr_pool = ctx.enter_context(tc.tile_pool(name="drs", bufs=4, space="DRAM"))

    streams = [
        dict(cs=IA, aug=False, scale=float(IA) ** -0.5, K=ka, Q=qa, VT=vta,
             WP=wpA, cpart=cA, res=f2d_r, out=io["x2d"], nproj=2),
        dict(cs=IB, aug=True, scale=float(IB) ** -0.5, K=kb, Q=qb, VT=vtb,
             WP=wpB, cpart=cB, res=f3d_r, out=io["x3d"], nproj=1),
    ]

    # Global software pipeline over (chunk, group): the AV lag carries across
    # chunk boundaries, so chunk c's last AV groups interleave with chunk
    # c+1's first QK/exp groups instead of draining + refilling the pipe.
    state = [dict(st=st, pend=[], o_ci=None) for st in streams]

    def emit_qk(s, ci, g):
        st = s["st"]
        n0 = ci * CHW
        lg = log_pool.tile([128, GR, CHW], F32, name="lg", tag="lg")
        for t in range(GR):
            tm = g * GR + t
            mm(lg[:, t, :], lhsT=st["K"][:, tm * MT:(tm + 1) * MT],
               rhs=st["Q"][:, n0:n0 + CHW], start=True, stop=True)
        E = e_pool.tile([128, GR, CHW], BF16, name="E", tag="E")
        nc.scalar.activation(E[:], lg[:], EXP, scale=st["scale"])
        s["pend"].append((ci, g, E))

    def emit_av(s):
        st = s["st"]
        mw = st["cs"] + 1 if st["aug"] else st["cs"]
        ci, g, E = s["pend"].pop(0)
        if s["o_ci"] != ci:
            s["o_ci"] = ci
            s["o_ps"] = o_pool.tile([128, CHW], F32, name="o_ps", tag="o")
            s["tree"] = {}
        for t in range(GR):
            tm = g * GR + t
            mm(s["o_ps"][0:mw, :], lhsT=st["VT"][:, tm, 0:mw], rhs=E[:, t, :],
               start=(tm == 0), stop=(tm == NMT - 1))
        if not st["aug"]:
            # stream-A denominators on the otherwise-idle DVE: pairwise bf16
            # adds (2x packed mode) cascading a binary tree over the E tiles
            l1 = wk_pool.tile([128, CHW], BF16, name="l1", tag="tl1")
            nc.vector.tensor_tensor(out=l1[:], in0=E[:, 0, :], in1=E[:, 1, :],
                                    op=ADD)
            tree = s["tree"]
            tree.setdefault(1, []).append(l1)
            lev = 1
            while len(tree.get(lev, [])) == 2:
                a, b = tree[lev]
                tree[lev] = []
                nxt = wk_pool.tile([128, CHW], BF16, name="lv",
                                   tag=f"tl{lev + 1}")
                nc.vector.tensor_tensor(out=nxt[:], in0=a[:], in1=b[:], op=ADD)
                tree.setdefault(lev + 1, []).append(nxt)
                lev += 1
        if g == NGR - 1:
            emit_tail(s, ci)

    def emit_tail(s, ci):
        n0 = ci * CHW
        st, o_ps = s["st"], s["o_ps"]
        cs = st["cs"]
        if True:
            # O to SBUF first: frees the PSUM slot for the next chunk before
            # the reciprocal/broadcast chain occupies the DVE
            o_sb = wk_pool.tile([128, CHW], BF16, name="o_sb", tag="osb")
            nc.vector.tensor_copy(out=o_sb[0:cs, :], in_=o_ps[0:cs, :])

            # ---- denominators: PE ones-matmul broadcast + direct recip ----
            bca = wk_pool.tile([128, CHW], F32, name="bca", tag="bca")
            if st["aug"]:
                drow = wk_pool.tile([1, CHW], BF16, name="drow", tag="drow")
                nc.vector.tensor_copy(out=drow[:], in_=o_ps[cs:cs + 1, :])
                d_ps = pj_pool.tile([128, CHW], F32, name="d_ps", tag="pj")
                mm(d_ps[:], lhsT=ones_row[:, 0:128], rhs=drow[:],
                   start=True, stop=True)
                nc.vector.reciprocal(bca[:], d_ps[:])
            else:
                # fold the bf16 tree root across all 128 partitions with one
                # all-ones matmul; output rows are all D, so the reciprocal
                # of the tile IS the broadcast 1/D
                d5 = s["tree"][5].pop()
                d_ps = pj_pool.tile([128, CHW], F32, name="d_ps", tag="pj")
                mm(d_ps[:], lhsT=ones_mat[:], rhs=d5[:], start=True, stop=True)
                nc.vector.reciprocal(bca[:], d_ps[:])

            # ---- projection + normalize + residual ----
            for k in range(st["nproj"]):
                p_ps = pj_pool.tile([128, CHW], F32, name="p_ps", tag="pj")
                mm(p_ps[:], lhsT=st["WP"][0:cs, k * 128:(k + 1) * 128],
                   rhs=o_sb[0:cs, :], start=True, stop=True)
                tmp = wk_pool.tile([128, CHW], F32, name="tmp", tag="tmp")
                nc.vector.tensor_tensor(out=tmp[:], in0=p_ps[:], in1=bca[:], op=MULT)
                outt = wk_pool.tile([128, CHW], F32, name="outt", tag="outt")
                if st["nproj"] == 2:
                    resid = st["res"][:, k, n0:n0 + CHW]
                else:
                    resid = st["res"][:, n0:n0 + CHW]
                nc.vector.scalar_tensor_tensor(
                    out=outt[:], in0=tmp[:], scalar=st["cpart"][:, k:k + 1],
                    in1=resid, op0=ADD, op1=ADD)
                nc.sync.dma_start(
                    out=st["out"][k * 128:(k + 1) * 128, n0:n0 + CHW], in_=outt[:])

    for ci in range(NCH):
        for gg in range(0, NGR, 2):
            for s in state:
                emit_qk(s, ci, gg)
                emit_qk(s, ci, gg + 1)
                while len(s["pend"]) > 2:
                    emit_av(s)
    for s in state:
        while s["pend"]:
            emit_av(s)


_BUILT = {}


def _get_built(with_bias=False):
    if with_bias not in _BUILT:
        nc = bacc.Bacc("TRN2", target_bir_lowering=False, debug=False)
        io = _declare_io(nc)
        with tile.TileContext(nc) as tc:
            with ExitStack() as ctx:
                _emit(ctx, tc, io, with_bias)
        nc.compile()
        _BUILT[with_bias] = nc
    return _BUILT[with_bias]


def make_in_maps(features_2d, features_3d,
                 wqA, bqA, wkA, bkA, wvA, bvA, wpA, bpA,
                 wqB, bqB, wkB, bkB, wvB, bvB, wpB, bpB):
    f32 = np.float32
    bf = ml_dtypes.bfloat16
    f2d = np.asarray(features_2d, f32).reshape(B, C2D, N)
    f3d = np.asarray(features_3d, f32).reshape(B, C3D, N)
    wqA, bqA = np.asarray(wqA, f32), np.asarray(bqA, f32)
    wkA, bkA = np.asarray(wkA, f32), np.asarray(bkA, f32)
    wvA, bvA = np.asarray(wvA, f32), np.asarray(bvA, f32)
    wpA, bpA = np.asarray(wpA, f32), np.asarray(bpA, f32)
    wqB, bqB = np.asarray(wqB, f32), np.asarray(bqB, f32)
    wkB, bkB = np.asarray(wkB, f32), np.asarray(bkB, f32)
    wvB, bvB = np.asarray(wvB, f32), np.asarray(bvB, f32)
    wpB, bpB = np.asarray(wpB, f32), np.asarray(bpB, f32)

    shared = {
        "wqA_T": np.ascontiguousarray(wqA.T).astype(bf),
        "wkA_T": np.ascontiguousarray(wkA.T).astype(bf),
        "wvA_T": np.ascontiguousarray(wvA.T).astype(bf),
        "wpA_T": np.ascontiguousarray(wpA.T).astype(bf),
        "wqB_T": np.ascontiguousarray(wqB.T).astype(bf),
        "wkB_T": np.ascontiguousarray(wkB.T).astype(bf),
        "wvB_T": np.ascontiguousarray(wvB.T).astype(bf),
        "wpB_T": np.ascontiguousarray(wpB.T).astype(bf),
        "bqA_r": bqA[None, :].astype(bf),
        "bkA_r": bkA[None, :].astype(bf),
        "bqB_r": bqB[None, :].astype(bf),
        "bkB_r": bkB[None, :].astype(bf),
        "cA": (wpA @ bvA + bpA).astype(f32),
        "cB": (wpB @ bvB + bpB).astype(f32),
    }
    in_maps = []
    for core in range(8):
        b, half = divmod(core, 2)
        if half == 0:
            p2, p3 = f2d[b], f3d[b]
        else:
            p2 = np.concatenate([f2d[b][:, NQ:], f2d[b][:, :NQ]], axis=1)
            p3 = np.concatenate([f3d[b][:, NQ:], f3d[b][:, :NQ]], axis=1)
        in_maps.append({
            "f2d_bf": np.ascontiguousarray(p2).astype(bf),
            "f3d_bf": np.ascontiguousarray(p3).astype(bf),
            "f2d_res": np.ascontiguousarray(p2[:, :NQ]),
            "f3d_res": np.ascontiguousarray(p3[:, :NQ]),
            **shared,
        })
    return in_maps


def assemble(results):
    x2d = np.empty((B, C2D, N), np.float32)
    x3d = np.empty((B, C3D, N), np.float32)
    for core in range(8):
        b, half = divmod(core, 2)
        x2d[b][:, half * NQ:(half + 1) * NQ] = results[core]["x2d"]
        x3d[b][:, half * NQ:(half + 1) * NQ] = results[core]["x3d"]
    return x2d.reshape(B, C2D, H, W), x3d.reshape(B, C3D, H, W)


def kernel(**inputs):
    with_bias = any(
        np.any(np.asarray(inputs[k], np.float32))
        for k in ("bqA", "bkA", "bqB", "bkB"))
    nc = _get_built(with_bias)
    in_maps = make_in_maps(**inputs)
    res = run_bass_kernel_spmd(nc, in_maps, list(range(8))).results
    return assemble(res)
